# revision 2
# baseline (speedup 1.0000x reference)
"""Morphological dilation (depthwise 3x3, additive SE) on 8 TRN2 NeuronCores.

out[b,c,h,w] = max_{dy,dx in {-1,0,1}} ( x[b,c,h+dy,w+dx] + k[c, (dy+1)*3+(dx+1)] )
with zero padding outside the image.

Sharding: batch -> 8 cores (1 image each). Per core, partitions = (h_half, c)
(2*64 = 128), free dim = rows x cols, processed in chunks of R=16 rows.

Math per chunk: 9 shifted adds (z_i = shift_i(x) + k_i) + 8 pairwise maxes.
All shifts are folded into the *add* stage reads. To keep every DVE op in its
fast mode (tensor_scalar 4x, tensor_tensor 2x: both need 4-byte-aligned fp16
APs), two column-parity copies of x are kept in SBUF:
  xe = padded cols [0..226)   -> dx=-1 reads at col 0, dx=+1 reads at col 2
  xo = raw    cols (=pad 1..225) -> dx=0 reads at col 0
so every access-pattern base lands on a 4-byte boundary.
"""

import numpy as np

_CACHE = {}

C = 64
H = 224
W = 224
R = 16          # output rows per chunk
HALF = 112      # rows per h-half
NCH = HALF // R
DVE_ADDS = (0, 4, 8)  # one add per column-group runs on VectorE; rest on ScalarE


def _build():
    import concourse.tile as tile
    import concourse.mybir as mybir
    from concourse import bacc

    f16 = mybir.dt.float16
    f32 = mybir.dt.float32

    nc = bacc.Bacc("TRN2", target_bir_lowering=False, debug=False)
    x_t = nc.dram_tensor("x", [C, H + 2, W + 2], f16, kind="ExternalInput")
    k_t = nc.dram_tensor("k", [128, 9], f32, kind="ExternalInput")
    o_t = nc.dram_tensor("out", [C, H, W], f16, kind="ExternalOutput")

    with tile.TileContext(nc) as tc:
        with (
            tc.tile_pool(name="const", bufs=1) as cpool,
            tc.tile_pool(name="xin", bufs=2) as xpool,
            tc.tile_pool(name="z", bufs=10) as zpool,
            tc.tile_pool(name="m", bufs=6) as mpool,
            tc.tile_pool(name="o", bufs=3) as opool,
        ):
            kb = cpool.tile([128, 9], f32)
            nc.sync.dma_start(kb[:], k_t[:])

            for ci in range(NCH):
                r0 = ci * R
                xe = xpool.tile([128, R + 2, W + 2], f16, tag="xe")
                xo = xpool.tile([128, R + 2, W], f16, tag="xo")
                for half in range(2):
                    rows = slice(half * HALF + r0, half * HALF + r0 + R + 2)
                    ps = slice(half * C, half * C + C)
                    nc.sync.dma_start(xe[ps, :, :], x_t[:, rows, :])
                    nc.sync.dma_start(xo[ps, :, :], x_t[:, rows, 1 : W + 1])

                zs = []
                for i in range(9):
                    dyp = i // 3  # row offset inside the haloed tile
                    col = i % 3
                    if col == 0:
                        src = xe[:, dyp : dyp + R, 0:W]
                    elif col == 1:
                        src = xo[:, dyp : dyp + R, 0:W]
                    else:
                        src = xe[:, dyp : dyp + R, 2 : W + 2]
                    z = zpool.tile([128, R, W], f16, tag="z")
                    kap = kb[:, i : i + 1]
                    if i in DVE_ADDS:
                        nc.vector.tensor_scalar_add(z[:], src, kap)
                    else:
                        nc.scalar.add(z[:], src, kap)
                    zs.append(z)

                ms = []
                for g in range(3):  # column groups dx=-1,0,+1
                    m = mpool.tile([128, R, W], f16, tag="m")
                    nc.vector.tensor_max(m[:], zs[g][:], zs[g + 3][:])
                    nc.vector.tensor_max(m[:], m[:], zs[g + 6][:])
                    ms.append(m)
                o = opool.tile([128, R, W], f16, tag="o")
                nc.vector.tensor_max(ms[0][:], ms[0][:], ms[2][:])
                nc.vector.tensor_max(o[:], ms[0][:], ms[1][:])

                for half in range(2):
                    rows = slice(half * HALF + r0, half * HALF + r0 + R)
                    ps = slice(half * C, half * C + C)
                    nc.sync.dma_start(o_t[:, rows, :], o[ps, :, :])
    nc.finalize()
    return nc


LAST_RESULT = None


def kernel(x, kernel):
    """x: [8,64,224,224] f32; kernel: [1,64,9,1,1] f32 -> [8,64,224,224] f32."""
    global LAST_RESULT
    from concourse.bass_utils import run_bass_kernel_spmd

    if "nc" not in _CACHE:
        _CACHE["nc"] = _build()
    nc = _CACHE["nc"]

    B = x.shape[0]
    xp = np.zeros((B, C, H + 2, W + 2), np.float16)
    xp[:, :, 1 : H + 1, 1 : W + 1] = x
    kb = np.ascontiguousarray(np.asarray(kernel, np.float32).reshape(C, 9))
    kb = np.concatenate([kb, kb], axis=0)  # [128, 9]; partition p = half*64 + c

    in_maps = [{"x": xp[b], "k": kb} for b in range(B)]
    res = run_bass_kernel_spmd(nc, in_maps, core_ids=list(range(B)))
    LAST_RESULT = res
    out = np.stack([r["out"] for r in res.results], axis=0)
    return out.astype(np.float32)
